# revision 20
# baseline (speedup 1.0000x reference)
"""Multi-headed attention kernel for 8 Trainium2 NeuronCores.

Problem: B=4, S=2048, E=1024, H=16, D=64 (torch-convention Linears, no bias
on q/k/v, bias on output projection).

Sharding: core c handles (batch b = c//2, query half sh = c%2).  Each core
computes Q for its 1024 query rows, K/V for the full 2048 keys of its batch
(duplicated across the pair of cores sharing a batch -- cheaper than any
cross-core collective), all 16 heads of attention for its rows, and the
output projection + bias.  Zero collectives.

Layout (feature dim on partitions; scores computed transposed):
  qT[f, q]  = sum_e WqT[e, f] * XT[e, q]          kT[f, s] likewise
  V[s, f]   = sum_e XT[e, s-chunk] * WvT[e, f]    (natural layout)
  scoresT[k, q] = sum_d kT[h*64+d, kc] * qT[h*64+d, q]    (K=64 matmuls,
      head pair packed in complementary PE row groups)
  EX = exp(scoresT / 8)            (ACT engine, PSUM -> SBUF bf16)
  ctxT_aug[m, q] = sum_k Vaug[k, h*65+m] * EX[k, q]   m in 0..64; V carries
      a ones column per head, so row 64 of the accumulation = softmax
      denominators (ones-column trick, M=65 matmuls)
  ctxT_norm = ctxT * bcast(1/denom)   (K=1 matmul broadcast of the
      denominator row + one-shot approx reciprocal on the DVE)
  out[s, e] = sum_f ctxT_norm[f, s-chunk] * WoT[f, e] + ones.T @ bo

Scheduling: the attention loop is ACT-bound (256 exp calls of [128,1024]).
Q/K/V projection work for head pair p+1 is emitted as "filler" inside pair
p's key-chunk loop so the PE stays dense while ACT streams exps (keeps the
HAM clock gate warm).  PSUM: scores 2x[128,1024] + ctx 3x[65,512] +
proj/bcast 1 = 8 banks.
"""

import os

import numpy as np
import ml_dtypes

import concourse.bass as bass
from concourse import bacc
import concourse.mybir as mybir
import concourse.tile as tile
from concourse.bass_utils import run_bass_kernel_spmd

B, S, E, H = 4, 2048, 1024, 16
D = E // H  # 64
P = 128
SL = S // 2     # local query rows per core (1024)
NCORES = 8
EC = E // P     # 8 e-chunks
FC = E // P     # 8 feature chunks
SC = S // P     # 16 s-chunks (V natural layout)
KC = S // P     # 16 key chunks (scores partition dim)
QB = SL // 512  # 2 query blocks of 512

F32 = mybir.dt.float32
F32R = mybir.dt.float32r
BF16 = mybir.dt.bfloat16
EXPF = mybir.ActivationFunctionType.Exp
NPBF = ml_dtypes.bfloat16

_CACHE = {}


def build():
    nc = bacc.Bacc(
        "TRN2",
        target_bir_lowering=False,
        debug=False,
        num_devices=NCORES,
    )

    xt_d = nc.dram_tensor("xt", [E, S], BF16, kind="ExternalInput").ap()
    # wq2/wk2 are host-pretiled: row fc*128+p, col ec*128+c  =  W.T[ec*128+p,
    # fc*128+c], so one contiguous [128, E] DMA delivers all 8 lhsT slices
    # for feature chunk fc.
    wq2_d = nc.dram_tensor("wq2", [E, E], BF16, kind="ExternalInput").ap()
    wk2_d = nc.dram_tensor("wk2", [E, E], BF16, kind="ExternalInput").ap()
    wvt_d = nc.dram_tensor("wvt", [E, E], BF16, kind="ExternalInput").ap()
    wot_d = nc.dram_tensor("wot", [E, E], BF16, kind="ExternalInput").ap()
    bo_d = nc.dram_tensor("bo", [1, E], BF16, kind="ExternalInput").ap()
    # one-hot selector rows: sel[r, r*64:(r+1)*64] = 1 (denominator broadcast)
    sel_d = nc.dram_tensor("sel", [4, 4 * D], F32R, kind="ExternalInput").ap()
    out_d = nc.dram_tensor("out", [SL, E], F32, kind="ExternalOutput").ap()

    with tile.TileContext(nc) as tc:
     with tc.tile_pool(name="persist", bufs=1) as persist:
        qt_sb = persist.tile([P, FC, SL], BF16, tag="qt")
        kt_sb = persist.tile([P, FC, S], BF16, tag="kt")
        DA = D + 1  # head dim + ones column
        vaug_sb = persist.tile([P, SC, H * DA], BF16, tag="vaug")
        vview = vaug_sb.rearrange("p c (h d) -> p c h d", d=DA)
        nc.vector.memset(vview[:, :, :, D : D + 1], 1.0)
        ctxt_sb = persist.tile([P, FC, SL], BF16, tag="ctxt")

        ones_bf = persist.tile([1, P], BF16, tag="ones_bf")   # bias matmul lhsT
        nc.vector.memset(ones_bf[:], 1.0)
        sel_sb = persist.tile([4, 4 * D], F32R, tag="sel")
        nc.sync.dma_start(out=sel_sb[:], in_=sel_d[:])

        from contextlib import ExitStack

        with (
            tc.tile_pool(name="wvp", bufs=8) as wvpool,
            tc.tile_pool(name="wqkp", bufs=5) as wqkpool,
            tc.tile_pool(name="expp", bufs=8) as exppool,
            tc.tile_pool(name="smallp", bufs=5) as smallpool,
        ):
            _xstack = ExitStack()
            _ostack = ExitStack()
            xpool = _xstack.enter_context(tc.tile_pool(name="xp", bufs=1))
            x_sb = xpool.tile([P, EC, S], BF16, tag="x")
            wv = []

            def load_wfc(w_dram, fc):
                """One [128, E] tile holding all 8 lhsT slices for chunk fc."""
                t = wqkpool.tile([P, E], BF16, tag="wqk", name="wqk")
                nc.sync.dma_start(out=t[:], in_=w_dram[fc * P : (fc + 1) * P, :])
                return t

            # ---- projection group emitters (8 accumulating MMs + 1 cast) ----
            def q_group(pool, wq_t, fc, qb):
                ps = pool.tile([P, 512], F32, tag="pj", name="pj")
                for ec in range(EC):
                    nc.tensor.matmul(
                        ps[:],
                        wq_t[:, ec * P : (ec + 1) * P],
                        x_sb[:, ec, qb * 512 : qb * 512 + 512],
                        start=(ec == 0),
                        stop=(ec == EC - 1),
                    )
                nc.vector.tensor_copy(
                    out=qt_sb[:, fc, qb * 512 : qb * 512 + 512], in_=ps[:]
                )

            def k_group(pool, wk_t, fc, kb):
                ps = pool.tile([P, 512], F32, tag="pj", name="pj")
                for ec in range(EC):
                    nc.tensor.matmul(
                        ps[:],
                        wk_t[:, ec * P : (ec + 1) * P],
                        x_sb[:, ec, kb * 512 : kb * 512 + 512],
                        start=(ec == 0),
                        stop=(ec == EC - 1),
                    )
                nc.vector.tensor_copy(
                    out=kt_sb[:, fc, kb * 512 : kb * 512 + 512], in_=ps[:]
                )

            def v_group(pool, sc, fb):
                ps = pool.tile([P, 512], F32, tag="pj", name="pj")
                for ec in range(EC):
                    nc.tensor.matmul(
                        ps[:],
                        x_sb[:, ec, sc * P : (sc + 1) * P],
                        wv[ec][:, fb * 512 : fb * 512 + 512],
                        start=(ec == 0),
                        stop=(ec == EC - 1),
                    )
                vv = vaug_sb[:, sc, :].rearrange("p (h d) -> p h d", d=DA)
                nc.vector.tensor_copy(
                    out=vv[:, fb * 8 : (fb + 1) * 8, 0:D],
                    in_=ps.rearrange("p (h d) -> p h d", d=D),
                )

            # ---------------- upfront: just enough for pair 0 ----------------
            # W chunk-0 tiles go on the Sync DMA queue; X streams in parallel
            # on the (otherwise idle) GpSimd DMA queue, Wv after it on Sync.
            wq_sl = load_wfc(wq2_d, 0)
            wk_sl = load_wfc(wk2_d, 0)
            for ec in range(EC):
                for hx in range(2):
                    nc.gpsimd.dma_start(
                        out=x_sb[:, ec, hx * 1024 : (hx + 1) * 1024],
                        in_=xt_d[ec * P : (ec + 1) * P, hx * 1024 : (hx + 1) * 1024],
                    )
            for ec in range(EC):
                t = wvpool.tile([P, E], BF16, tag="wv", name="wv")
                nc.sync.dma_start(out=t[:], in_=wvt_d[ec * P : (ec + 1) * P, :])
                wv.append(t)
            with tc.tile_pool(name="psum_u", bufs=6, space="PSUM") as psum_u:
                # advance all 6 Q/K accumulation groups together per arriving
                # X chunk: each 1.6us chunk DMA feeds ~1.6us of matmuls, so
                # the PE ramps with the DMA stream instead of stalling on the
                # last chunk of each group.
                psq = [
                    psum_u.tile([P, 512], F32, tag="pj", name="pj")
                    for _ in range(QB)
                ]
                psk = [
                    psum_u.tile([P, 512], F32, tag="pj", name="pj")
                    for _ in range(4)
                ]
                for ec in range(EC):
                    for qb in range(QB):
                        nc.tensor.matmul(
                            psq[qb][:],
                            wq_sl[:, ec * P : (ec + 1) * P],
                            x_sb[:, ec, qb * 512 : qb * 512 + 512],
                            start=(ec == 0),
                            stop=(ec == EC - 1),
                        )
                    for kb in range(4):
                        nc.tensor.matmul(
                            psk[kb][:],
                            wk_sl[:, ec * P : (ec + 1) * P],
                            x_sb[:, ec, kb * 512 : kb * 512 + 512],
                            start=(ec == 0),
                            stop=(ec == EC - 1),
                        )
                for qb in range(QB):
                    nc.vector.tensor_copy(
                        out=qt_sb[:, 0, qb * 512 : qb * 512 + 512], in_=psq[qb][:]
                    )
                for kb in range(4):
                    nc.vector.tensor_copy(
                        out=kt_sb[:, 0, kb * 512 : kb * 512 + 512], in_=psk[kb][:]
                    )
                # only V for the first few key chunks upfront; the rest are
                # produced inside pair 0's first kc loop (consumption of
                # vaug[sc] starts at kc==sc, so production stays ahead while
                # the exp stream hides the PE cost)
                for sc in range(4):
                    v_group(psum_u, sc, 0)

            # ---------------- pair loop ----------------
            # Emission-order = scheduler priority.  The kc loop is emitted
            # first (scores/exp/ctx only); filler projections for pair fc+1
            # are emitted AFTER it, so the priority heap interleaves filler
            # MMs into PE slack at single-MM granularity without ever
            # delaying a ready scores MM (they outrank the fillers).
            wot_t = []
            bo_sb = None
            with (
                tc.tile_pool(name="psum_sc", bufs=2, space="PSUM") as psum_sc,
                tc.tile_pool(name="psum_cx", bufs=2, space="PSUM") as psum_cx,
                tc.tile_pool(name="psum_pj", bufs=2, space="PSUM") as psum_pj,
            ):
                for fc in range(FC):
                    hA, hB = 2 * fc, 2 * fc + 1
                    for qb in range(QB):
                        ctx_ps = {
                            hh: psum_cx.tile([DA, 512], F32, tag="ctx", name="ctx")
                            for hh in (0, 1)
                        }
                        for kc in range(KC):
                            with tc.high_priority(offset=1 << 20):
                                sc_ps = psum_sc.tile(
                                    [P, 1024], F32, tag="sc", name="sc"
                                )
                                for hh, h in ((0, hA), (1, hB)):
                                    po = hh * D
                                    nc.tensor.matmul(
                                        sc_ps[:, hh * 512 : hh * 512 + 512],
                                        kt_sb[
                                            po : po + D, fc, kc * P : (kc + 1) * P
                                        ],
                                        qt_sb[
                                            po : po + D,
                                            fc,
                                            qb * 512 : qb * 512 + 512,
                                        ],
                                        start=True,
                                        stop=True,
                                    )
                                ex = exppool.tile(
                                    [P, 1024], BF16, tag="exp", name="exp"
                                )
                                nc.scalar.activation(
                                    ex[:], sc_ps[:], EXPF, scale=0.125
                                )
                                for hh, h in ((0, hA), (1, hB)):
                                    # ctx + denom row via ones column (M=65)
                                    nc.tensor.matmul(
                                        ctx_ps[hh][0:DA, :],
                                        vaug_sb[:, kc, h * DA : (h + 1) * DA],
                                        ex[:, hh * 512 : hh * 512 + 512],
                                        start=(kc == 0),
                                        stop=(kc == KC - 1),
                                    )
                            if fc == 0 and qb == 0 and kc < SC - 4:
                                # stream the remaining pair-0 V chunks; the
                                # exp pipeline (exppool depth 8) absorbs the
                                # ctx lag while these run in PE slack
                                v_group(psum_pj, kc + 4, 0)

                        # ---- normalize: ctxt = ctx * bcast(1/denom) ----
                        # den extraction + ctxt copy free the ctx PSUM banks
                        # for the next qb -> keep them ahead of filler CASTs
                        # on the DVE queue.
                        with tc.high_priority(offset=1 << 20):
                            dens = []
                            for hh in (0, 1):
                                den = smallpool.tile(
                                    [1, 512], F32R, tag="den", name="den"
                                )
                                nc.vector.tensor_copy(
                                    out=den[:], in_=ctx_ps[hh][D : D + 1, :]
                                )
                                dens.append(den)
                            # release ctx banks: unnormalized bf16 into ctxt
                            for hh in (0, 1):
                                nc.vector.tensor_copy(
                                    out=ctxt_sb[
                                        hh * D : (hh + 1) * D,
                                        fc,
                                        qb * 512 : qb * 512 + 512,
                                    ],
                                    in_=ctx_ps[hh][0:D, :],
                                )

                        def _norm(dens=dens, fc=fc, qb=qb):
                            rec = smallpool.tile(
                                [P, 512], F32, tag="rec", name="rec"
                            )
                            for hh in (0, 1):
                                bc_ps = psum_pj.tile(
                                    [P, 512], F32, tag="pj", name="pj"
                                )
                                nc.tensor.matmul(
                                    bc_ps[0:D, :],
                                    sel_sb[0:1, 0:D],
                                    dens[hh][:],
                                    start=True,
                                    stop=True,
                                )
                                if hh == 0:
                                    nc.vector.reciprocal_approx_fast(
                                        out=rec[0:D, :], in_=bc_ps[0:D, :]
                                    )
                                else:
                                    # approx recip can't shift partitions; recip
                                    # at base 0 then cross-partition copy
                                    rtmp = smallpool.tile(
                                        [D, 512], F32, tag="rtmp", name="rtmp"
                                    )
                                    nc.vector.reciprocal_approx_fast(
                                        out=rtmp[:], in_=bc_ps[0:D, :]
                                    )
                                    nc.vector.tensor_copy(
                                        out=rec[D : 2 * D, :], in_=rtmp[:]
                                    )
                            dst = ctxt_sb[:, fc, qb * 512 : qb * 512 + 512]
                            nc.vector.tensor_mul(out=dst, in0=dst, in1=rec[:])

                        _norm()
                        if fc == FC - 1:
                            # overlap the output projection for this qb's
                            # rows with the remaining attention work (uses
                            # the pj PSUM ring as filler-priority MMs)
                            for sc in range(qb * 4, qb * 4 + 4):
                                ot = outpool.tile([P, E], F32, tag="out", name="out")
                                for eb in range(2):
                                    ps = psum_pj.tile(
                                        [P, 512], F32, tag="pj", name="pj"
                                    )
                                    for fcc in range(FC):
                                        nc.tensor.matmul(
                                            ps[:],
                                            ctxt_sb[:, fcc, sc * P : (sc + 1) * P],
                                            wot_t[fcc][
                                                :, eb * 512 : eb * 512 + 512
                                            ],
                                            start=(fcc == 0),
                                            stop=False,
                                        )
                                    nc.tensor.matmul(
                                        ps[:],
                                        ones_bf[:],
                                        bo_sb[:, eb * 512 : eb * 512 + 512],
                                        start=False,
                                        stop=True,
                                    )
                                    nc.vector.tensor_copy(
                                        out=ot[:, eb * 512 : eb * 512 + 512],
                                        in_=ps[:],
                                    )
                                nc.sync.dma_start(
                                    out=out_d[sc * P : (sc + 1) * P, :], in_=ot[:]
                                )

                    # ---- fillers: prep pair fc+1 (emitted after the kc
                    # loop so every in-loop MM outranks them) ----
                    if fc + 1 < FC:
                        nfc = fc + 1
                        wq_n = load_wfc(wq2_d, nfc)
                        wk_n = load_wfc(wk2_d, nfc)
                        for qb in range(QB):
                            q_group(psum_pj, wq_n, nfc, qb)
                        for kb in range(4):
                            k_group(psum_pj, wk_n, nfc, kb)
                        if fc < 4:
                            for sc in range(4 * fc, 4 * fc + 4):
                                v_group(psum_pj, sc, 1)
                    if fc == 4:
                        # prefetch Wo/bo into the (now dead) wv slots so the
                        # output projection never waits on DMA
                        for fcc in range(FC):
                            t = wvpool.tile([P, E], BF16, tag="wv", name="wv")
                            nc.sync.dma_start(
                                out=t[:], in_=wot_d[fcc * P : (fcc + 1) * P, :]
                            )
                            wot_t.append(t)
                        bo_sb = persist.tile([1, E], BF16, tag="bo")
                        nc.sync.dma_start(out=bo_sb[:], in_=bo_d[:])
                    if fc == FC - 2:
                        # X is dead once pair-7 prep is done; free its SBUF
                        # and open the output staging pool in the freed space
                        _xstack.close()
                        outpool = _ostack.enter_context(
                            tc.tile_pool(name="outp", bufs=2)
                        )
            _ostack.close()

    nc.compile()
    return nc


def _tile_wfc(wt):
    """Pre-tile W.T so chunk fc's 8 lhsT slices are one contiguous row-block:
    out[fc*128+p, ec*128+c] = wt[ec*128+p, fc*128+c]."""
    a = wt.reshape(EC, P, FC, P).transpose(2, 1, 0, 3)
    return np.ascontiguousarray(a.reshape(FC * P, E))


def _prep_inputs(X, Wq, Wk, Wv, Wo, bo):
    X = np.asarray(X, dtype=np.float32)
    wqt = np.ascontiguousarray(np.asarray(Wq, np.float32).T).astype(NPBF)
    wkt = np.ascontiguousarray(np.asarray(Wk, np.float32).T).astype(NPBF)
    wq2 = _tile_wfc(wqt)
    wk2 = _tile_wfc(wkt)
    wvt = np.ascontiguousarray(np.asarray(Wv, np.float32).T).astype(NPBF)
    wot = np.ascontiguousarray(np.asarray(Wo, np.float32).T).astype(NPBF)
    bo2 = np.asarray(bo, np.float32).reshape(1, E).astype(NPBF)
    sel = np.zeros((4, 4 * D), np.float32)
    for r in range(4):
        sel[r, r * D : (r + 1) * D] = 1.0

    in_maps = []
    for c in range(NCORES):
        b, sh = c // 2, c % 2
        xt = np.ascontiguousarray(X[b].T)  # [E, S]
        if sh == 1:  # rotate so the local query half comes first
            xt = np.concatenate([xt[:, SL:], xt[:, :SL]], axis=1)
        in_maps.append(
            {
                "xt": np.ascontiguousarray(xt.astype(NPBF)),
                "wq2": wq2,
                "wk2": wk2,
                "wvt": wvt,
                "wot": wot,
                "bo": bo2,
                "sel": sel,
            }
        )
    return in_maps


LAST_EXEC_NS = None
LAST_RESULTS = None


def _ensure_ntff_hook_importable():
    """bass_utils imports antenv.axon_hooks when tracing is requested (e.g.
    BASS_TRACE=1 in the environment).  The RL container's antenv stub lacks
    that module; register a no-op fallback so tracing degrades gracefully
    instead of crashing.  If a real antenv.axon_hooks exists, do nothing."""
    import sys
    import types

    try:
        import antenv.axon_hooks  # noqa: F401

        return
    except ImportError:
        pass
    try:
        import antenv

        mod = types.ModuleType("antenv.axon_hooks")
        _hook = [None]
        mod.set_axon_ntff_profile_hook = lambda h: _hook.__setitem__(0, h)
        mod.get_axon_ntff_profile_hook = lambda: _hook[0]
        sys.modules["antenv.axon_hooks"] = mod
        antenv.axon_hooks = mod
        try:
            from trn_agent_boot.trn_boot import _ntff_profile_via_ctypes

            mod.set_axon_ntff_profile_hook(
                _ntff_profile_via_ctypes("/opt/axon/libaxon_pjrt.so")
            )
        except Exception:
            pass
    except Exception:
        pass


def _run(in_maps, trace=False):
    global LAST_EXEC_NS, LAST_RESULTS
    _ensure_ntff_hook_importable()
    if "nc" not in _CACHE:
        _CACHE["nc"] = build()
    res = run_bass_kernel_spmd(
        _CACHE["nc"],
        in_maps,
        core_ids=list(range(NCORES)),
        trace=trace,
    )
    LAST_RESULTS = res
    LAST_EXEC_NS = res.exec_time_ns
    return res


def kernel(X, Wq, Wk, Wv, Wo, bo):
    in_maps = _prep_inputs(X, Wq, Wk, Wv, Wo, bo)
    res = _run(in_maps, trace=bool(int(os.environ.get("KERNEL_TRACE", "0"))))
    out = np.empty((B, S, E), np.float32)
    for c in range(NCORES):
        b, sh = c // 2, c % 2
        out[b, sh * SL : (sh + 1) * SL, :] = res.results[c]["out"]
    return out



# revision 23
# speedup vs baseline: 1.0516x; 1.0516x over previous
"""Multi-headed attention kernel for 8 Trainium2 NeuronCores.

Problem: B=4, S=2048, E=1024, H=16, D=64 (torch-convention Linears, no bias
on q/k/v, bias on output projection).

Sharding: core c handles (batch b = c//2, query half sh = c%2).  Each core
computes Q for its 1024 query rows, K/V for the full 2048 keys of its batch
(duplicated across the pair of cores sharing a batch -- cheaper than any
cross-core collective), all 16 heads of attention for its rows, and the
output projection + bias.  Zero collectives.

Layout (feature dim on partitions; scores computed transposed):
  qT[f, q]  = sum_e WqT[e, f] * XT[e, q]          kT[f, s] likewise
  V[s, f]   = sum_e XT[e, s-chunk] * WvT[e, f]    (natural layout)
  scoresT[k, q] = sum_d kT[h*64+d, kc] * qT[h*64+d, q]    (K=64 matmuls,
      head pair packed in complementary PE row groups)
  EX = exp(scoresT / 8)            (ACT engine, PSUM -> SBUF bf16)
  ctxT_aug[m, q] = sum_k Vaug[k, h*65+m] * EX[k, q]   m in 0..64; V carries
      a ones column per head, so row 64 of the accumulation = softmax
      denominators (ones-column trick, M=65 matmuls)
  ctxT_norm = ctxT * bcast(1/denom)   (K=1 matmul broadcast of the
      denominator row + one-shot approx reciprocal on the DVE)
  out[s, e] = sum_f ctxT_norm[f, s-chunk] * WoT[f, e] + ones.T @ bo

Scheduling: the attention loop is ACT-bound (256 exp calls of [128,1024]).
Q/K/V projection work for head pair p+1 is emitted as "filler" inside pair
p's key-chunk loop so the PE stays dense while ACT streams exps (keeps the
HAM clock gate warm).  PSUM: scores 2x[128,1024] + ctx 3x[65,512] +
proj/bcast 1 = 8 banks.
"""

import os

import numpy as np
import ml_dtypes

import concourse.bass as bass
from concourse import bacc
import concourse.mybir as mybir
import concourse.tile as tile
from concourse.bass_utils import run_bass_kernel_spmd

B, S, E, H = 4, 2048, 1024, 16
D = E // H  # 64
P = 128
SL = S // 2     # local query rows per core (1024)
NCORES = 8
EC = E // P     # 8 e-chunks
FC = E // P     # 8 feature chunks
SC = S // P     # 16 s-chunks (V natural layout)
KC = S // P     # 16 key chunks (scores partition dim)
QB = SL // 512  # 2 query blocks of 512

F32 = mybir.dt.float32
F32R = mybir.dt.float32r
BF16 = mybir.dt.bfloat16
EXPF = mybir.ActivationFunctionType.Exp
NPBF = ml_dtypes.bfloat16

_CACHE = {}


def build():
    nc = bacc.Bacc(
        "TRN2",
        target_bir_lowering=False,
        debug=False,
        num_devices=NCORES,
    )

    xt_d = nc.dram_tensor("xt", [E, S], BF16, kind="ExternalInput").ap()
    # wq2/wk2 are host-pretiled: row fc*128+p, col ec*128+c  =  W.T[ec*128+p,
    # fc*128+c], so one contiguous [128, E] DMA delivers all 8 lhsT slices
    # for feature chunk fc.
    wq2_d = nc.dram_tensor("wq2", [E, E], BF16, kind="ExternalInput").ap()
    wk2_d = nc.dram_tensor("wk2", [E, E], BF16, kind="ExternalInput").ap()
    wvt_d = nc.dram_tensor("wvt", [E, E], BF16, kind="ExternalInput").ap()
    wot_d = nc.dram_tensor("wot", [E, E], BF16, kind="ExternalInput").ap()
    bo_d = nc.dram_tensor("bo", [1, E], BF16, kind="ExternalInput").ap()
    # one-hot selector rows: sel[r, r*64:(r+1)*64] = 1 (denominator broadcast)
    sel_d = nc.dram_tensor("sel", [4, 4 * D], F32R, kind="ExternalInput").ap()
    out_d = nc.dram_tensor("out", [SL, E], F32, kind="ExternalOutput").ap()

    with tile.TileContext(nc) as tc:
     with tc.tile_pool(name="persist", bufs=1) as persist:
        qt_sb = persist.tile([P, FC, SL], BF16, tag="qt")
        kt_sb = persist.tile([P, FC, S], BF16, tag="kt")
        DA = D + 1  # head dim + ones column
        vaug_sb = persist.tile([P, SC, H * DA], BF16, tag="vaug")
        vview = vaug_sb.rearrange("p c (h d) -> p c h d", d=DA)
        nc.vector.memset(vview[:, :, :, D : D + 1], 1.0)
        ctxt_sb = persist.tile([P, FC, SL], BF16, tag="ctxt")

        ones_bf = persist.tile([1, P], BF16, tag="ones_bf")   # bias matmul lhsT
        nc.vector.memset(ones_bf[:], 1.0)
        sel_sb = persist.tile([4, 4 * D], F32R, tag="sel")
        nc.sync.dma_start(out=sel_sb[:], in_=sel_d[:])

        from contextlib import ExitStack

        with (
            tc.tile_pool(name="wvp", bufs=8) as wvpool,
            tc.tile_pool(name="wqkp", bufs=5) as wqkpool,
            tc.tile_pool(name="expp", bufs=8) as exppool,
            tc.tile_pool(name="smallp", bufs=5) as smallpool,
        ):
            _xstack = ExitStack()
            _ostack = ExitStack()
            xpool = _xstack.enter_context(tc.tile_pool(name="xp", bufs=1))
            x_sb = xpool.tile([P, EC, S], BF16, tag="x")
            wv = []

            def load_wfc(w_dram, fc):
                """One [128, E] tile holding all 8 lhsT slices for chunk fc."""
                t = wqkpool.tile([P, E], BF16, tag="wqk", name="wqk")
                nc.sync.dma_start(out=t[:], in_=w_dram[fc * P : (fc + 1) * P, :])
                return t

            # ---- projection group emitters (8 accumulating MMs + 1 cast) ----
            def q_group(pool, wq_t, fc, qb):
                ps = pool.tile([P, 512], F32, tag="pj", name="pj")
                for ec in range(EC):
                    nc.tensor.matmul(
                        ps[:],
                        wq_t[:, ec * P : (ec + 1) * P],
                        x_sb[:, ec, qb * 512 : qb * 512 + 512],
                        start=(ec == 0),
                        stop=(ec == EC - 1),
                    )
                nc.vector.tensor_copy(
                    out=qt_sb[:, fc, qb * 512 : qb * 512 + 512], in_=ps[:]
                )

            def k_group(pool, wk_t, fc, kb):
                ps = pool.tile([P, 512], F32, tag="pj", name="pj")
                for ec in range(EC):
                    nc.tensor.matmul(
                        ps[:],
                        wk_t[:, ec * P : (ec + 1) * P],
                        x_sb[:, ec, kb * 512 : kb * 512 + 512],
                        start=(ec == 0),
                        stop=(ec == EC - 1),
                    )
                nc.vector.tensor_copy(
                    out=kt_sb[:, fc, kb * 512 : kb * 512 + 512], in_=ps[:]
                )

            def v_group(pool, sc, fb):
                ps = pool.tile([P, 512], F32, tag="pj", name="pj")
                for ec in range(EC):
                    nc.tensor.matmul(
                        ps[:],
                        x_sb[:, ec, sc * P : (sc + 1) * P],
                        wv[ec][:, fb * 512 : fb * 512 + 512],
                        start=(ec == 0),
                        stop=(ec == EC - 1),
                    )
                vv = vaug_sb[:, sc, :].rearrange("p (h d) -> p h d", d=DA)
                nc.vector.tensor_copy(
                    out=vv[:, fb * 8 : (fb + 1) * 8, 0:D],
                    in_=ps.rearrange("p (h d) -> p h d", d=D),
                )

            # ---------------- upfront: just enough for pair 0 ----------------
            # W chunk-0 tiles go on the Sync DMA queue; X streams in parallel
            # on the (otherwise idle) GpSimd DMA queue, Wv after it on Sync.
            wq_sl = load_wfc(wq2_d, 0)
            wk_sl = load_wfc(wk2_d, 0)
            for ec in range(EC):
                for hx in range(2):
                    nc.gpsimd.dma_start(
                        out=x_sb[:, ec, hx * 1024 : (hx + 1) * 1024],
                        in_=xt_d[ec * P : (ec + 1) * P, hx * 1024 : (hx + 1) * 1024],
                    )
            for ec in range(EC):
                t = wvpool.tile([P, E], BF16, tag="wv", name="wv")
                nc.sync.dma_start(out=t[:], in_=wvt_d[ec * P : (ec + 1) * P, :])
                wv.append(t)
            with tc.tile_pool(name="psum_u", bufs=6, space="PSUM") as psum_u:
                # advance all 6 Q/K accumulation groups together per arriving
                # X chunk: each 1.6us chunk DMA feeds ~1.6us of matmuls, so
                # the PE ramps with the DMA stream instead of stalling on the
                # last chunk of each group.
                psq = [
                    psum_u.tile([P, 512], F32, tag="pj", name="pj")
                    for _ in range(QB)
                ]
                psk = [
                    psum_u.tile([P, 512], F32, tag="pj", name="pj")
                    for _ in range(4)
                ]
                for ec in range(EC):
                    for qb in range(QB):
                        nc.tensor.matmul(
                            psq[qb][:],
                            wq_sl[:, ec * P : (ec + 1) * P],
                            x_sb[:, ec, qb * 512 : qb * 512 + 512],
                            start=(ec == 0),
                            stop=(ec == EC - 1),
                        )
                    for kb in range(4):
                        nc.tensor.matmul(
                            psk[kb][:],
                            wk_sl[:, ec * P : (ec + 1) * P],
                            x_sb[:, ec, kb * 512 : kb * 512 + 512],
                            start=(ec == 0),
                            stop=(ec == EC - 1),
                        )
                for qb in range(QB):
                    nc.vector.tensor_copy(
                        out=qt_sb[:, 0, qb * 512 : qb * 512 + 512], in_=psq[qb][:]
                    )
                for kb in range(4):
                    nc.vector.tensor_copy(
                        out=kt_sb[:, 0, kb * 512 : kb * 512 + 512], in_=psk[kb][:]
                    )
                # only V for the first few key chunks upfront; the rest are
                # produced inside pair 0's first kc loop (consumption of
                # vaug[sc] starts at kc==sc, so production stays ahead while
                # the exp stream hides the PE cost)
                for sc in range(4):
                    v_group(psum_u, sc, 0)

            # ---------------- pair loop ----------------
            # Emission-order = scheduler priority.  The kc loop is emitted
            # first (scores/exp/ctx only); filler projections for pair fc+1
            # are emitted AFTER it, so the priority heap interleaves filler
            # MMs into PE slack at single-MM granularity without ever
            # delaying a ready scores MM (they outrank the fillers).
            wot_t = []
            bo_sb = None
            with (
                tc.tile_pool(name="psum_sc", bufs=2, space="PSUM") as psum_sc,
                tc.tile_pool(name="psum_cx", bufs=2, space="PSUM") as psum_cx,
                tc.tile_pool(name="psum_pj", bufs=2, space="PSUM") as psum_pj,
            ):
                pending_norms = []
                for fc in range(FC):
                    hA, hB = 2 * fc, 2 * fc + 1
                    for qb in range(QB):
                        ctx_ps = {
                            hh: psum_cx.tile([DA, 512], F32, tag="ctx", name="ctx")
                            for hh in (0, 1)
                        }
                        for kc in range(KC):
                            with tc.high_priority(offset=1 << 20):
                                sc_ps = psum_sc.tile(
                                    [P, 1024], F32, tag="sc", name="sc"
                                )
                                for hh, h in ((0, hA), (1, hB)):
                                    po = hh * D
                                    nc.tensor.matmul(
                                        sc_ps[:, hh * 512 : hh * 512 + 512],
                                        kt_sb[
                                            po : po + D, fc, kc * P : (kc + 1) * P
                                        ],
                                        qt_sb[
                                            po : po + D,
                                            fc,
                                            qb * 512 : qb * 512 + 512,
                                        ],
                                        start=True,
                                        stop=True,
                                    )
                                ex = exppool.tile(
                                    [P, 1024], BF16, tag="exp", name="exp"
                                )
                                nc.scalar.activation(
                                    ex[:], sc_ps[:], EXPF, scale=0.125
                                )
                                for hh, h in ((0, hA), (1, hB)):
                                    # ctx + denom row via ones column (M=65)
                                    nc.tensor.matmul(
                                        ctx_ps[hh][0:DA, :],
                                        vaug_sb[:, kc, h * DA : (h + 1) * DA],
                                        ex[:, hh * 512 : hh * 512 + 512],
                                        start=(kc == 0),
                                        stop=(kc == KC - 1),
                                    )
                            if fc == 0 and qb == 0 and kc < SC - 4:
                                # stream the remaining pair-0 V chunks; the
                                # exp pipeline (exppool depth 8) absorbs the
                                # ctx lag while these run in PE slack
                                v_group(psum_pj, kc + 4, 0)

                        # ---- normalize: ctxt = ctx * bcast(1/denom) ----
                        # den extraction + ctxt copy free the ctx PSUM banks
                        # for the next qb -> keep them ahead of filler CASTs
                        # on the DVE queue.
                        with tc.high_priority(offset=1 << 20):
                            dens = []
                            for hh in (0, 1):
                                den = smallpool.tile(
                                    [1, 512], F32R, tag="den", name="den"
                                )
                                nc.vector.tensor_copy(
                                    out=den[:], in_=ctx_ps[hh][D : D + 1, :]
                                )
                                dens.append(den)
                            # release ctx banks: unnormalized bf16 into ctxt
                            for hh in (0, 1):
                                nc.vector.tensor_copy(
                                    out=ctxt_sb[
                                        hh * D : (hh + 1) * D,
                                        fc,
                                        qb * 512 : qb * 512 + 512,
                                    ],
                                    in_=ctx_ps[hh][0:D, :],
                                )

                        def _norm(dens=dens, fc=fc, qb=qb):
                            rec = smallpool.tile(
                                [P, 512], F32, tag="rec", name="rec"
                            )
                            for hh in (0, 1):
                                bc_ps = psum_pj.tile(
                                    [P, 512], F32, tag="pj", name="pj"
                                )
                                nc.tensor.matmul(
                                    bc_ps[0:D, :],
                                    sel_sb[0:1, 0:D],
                                    dens[hh][:],
                                    start=True,
                                    stop=True,
                                )
                                if hh == 0:
                                    nc.vector.reciprocal_approx_fast(
                                        out=rec[0:D, :], in_=bc_ps[0:D, :]
                                    )
                                else:
                                    # approx recip can't shift partitions; recip
                                    # at base 0 then cross-partition copy
                                    rtmp = smallpool.tile(
                                        [D, 512], F32, tag="rtmp", name="rtmp"
                                    )
                                    nc.vector.reciprocal_approx_fast(
                                        out=rtmp[:], in_=bc_ps[0:D, :]
                                    )
                                    nc.vector.tensor_copy(
                                        out=rec[D : 2 * D, :], in_=rtmp[:]
                                    )
                            dst = ctxt_sb[:, fc, qb * 512 : qb * 512 + 512]
                            nc.vector.tensor_mul(out=dst, in0=dst, in1=rec[:])

                        # Defer the norm (bcast MM + recip/mul chain): its
                        # sparse PE pattern would cool the HAM clock gate if
                        # it ran at an fc boundary; drained later it fills
                        # loop slack instead.  The last pair needs immediate
                        # norms for the overlapped output projection.
                        if fc < FC - 1:
                            pending_norms.append(_norm)
                        else:
                            _norm()
                        if fc == FC - 1:
                            # overlap the output projection for this qb's
                            # rows with the remaining attention work (uses
                            # the pj PSUM ring as filler-priority MMs)
                            for sc in range(qb * 4, qb * 4 + 4):
                                ot = outpool.tile([P, E], F32, tag="out", name="out")
                                for eb in range(2):
                                    ps = psum_pj.tile(
                                        [P, 512], F32, tag="pj", name="pj"
                                    )
                                    for fcc in range(FC):
                                        nc.tensor.matmul(
                                            ps[:],
                                            ctxt_sb[:, fcc, sc * P : (sc + 1) * P],
                                            wot_t[fcc][
                                                :, eb * 512 : eb * 512 + 512
                                            ],
                                            start=(fcc == 0),
                                            stop=False,
                                        )
                                    nc.tensor.matmul(
                                        ps[:],
                                        ones_bf[:],
                                        bo_sb[:, eb * 512 : eb * 512 + 512],
                                        start=False,
                                        stop=True,
                                    )
                                    nc.vector.tensor_copy(
                                        out=ot[:, eb * 512 : eb * 512 + 512],
                                        in_=ps[:],
                                    )
                                nc.sync.dma_start(
                                    out=out_d[sc * P : (sc + 1) * P, :], in_=ot[:]
                                )

                    # ---- fillers: prep pair fc+1 (emitted after the kc
                    # loop so every in-loop MM outranks them).  Hard deps of
                    # fc+1's first scores (K kb0, Q qb0) first; V groups and
                    # deferred norms spill harmlessly into fc+1's slack.
                    if fc + 1 < FC:
                        nfc = fc + 1
                        wk_n = load_wfc(wk2_d, nfc)
                        wq_n = load_wfc(wq2_d, nfc)
                        k_group(psum_pj, wk_n, nfc, 0)
                        q_group(psum_pj, wq_n, nfc, 0)
                        for kb in range(1, 4):
                            k_group(psum_pj, wk_n, nfc, kb)
                        q_group(psum_pj, wq_n, nfc, 1)
                        if fc < 4:
                            for sc in range(4 * fc, 4 * fc + 4):
                                v_group(psum_pj, sc, 1)
                    while pending_norms:
                        pending_norms.pop(0)()
                    if fc == 4:
                        # prefetch Wo/bo into the (now dead) wv slots so the
                        # output projection never waits on DMA
                        for fcc in range(FC):
                            t = wvpool.tile([P, E], BF16, tag="wv", name="wv")
                            nc.sync.dma_start(
                                out=t[:], in_=wot_d[fcc * P : (fcc + 1) * P, :]
                            )
                            wot_t.append(t)
                        bo_sb = persist.tile([1, E], BF16, tag="bo")
                        nc.sync.dma_start(out=bo_sb[:], in_=bo_d[:])
                    if fc == FC - 2:
                        # X is dead once pair-7 prep is done; free its SBUF
                        # and open the output staging pool in the freed space
                        _xstack.close()
                        outpool = _ostack.enter_context(
                            tc.tile_pool(name="outp", bufs=2)
                        )
            _ostack.close()

    nc.compile()
    return nc


def _tile_wfc(wt):
    """Pre-tile W.T so chunk fc's 8 lhsT slices are one contiguous row-block:
    out[fc*128+p, ec*128+c] = wt[ec*128+p, fc*128+c]."""
    a = wt.reshape(EC, P, FC, P).transpose(2, 1, 0, 3)
    return np.ascontiguousarray(a.reshape(FC * P, E))


def _prep_inputs(X, Wq, Wk, Wv, Wo, bo):
    X = np.asarray(X, dtype=np.float32)
    wqt = np.ascontiguousarray(np.asarray(Wq, np.float32).T).astype(NPBF)
    wkt = np.ascontiguousarray(np.asarray(Wk, np.float32).T).astype(NPBF)
    wq2 = _tile_wfc(wqt)
    wk2 = _tile_wfc(wkt)
    wvt = np.ascontiguousarray(np.asarray(Wv, np.float32).T).astype(NPBF)
    wot = np.ascontiguousarray(np.asarray(Wo, np.float32).T).astype(NPBF)
    bo2 = np.asarray(bo, np.float32).reshape(1, E).astype(NPBF)
    sel = np.zeros((4, 4 * D), np.float32)
    for r in range(4):
        sel[r, r * D : (r + 1) * D] = 1.0

    in_maps = []
    for c in range(NCORES):
        b, sh = c // 2, c % 2
        xt = np.ascontiguousarray(X[b].T)  # [E, S]
        if sh == 1:  # rotate so the local query half comes first
            xt = np.concatenate([xt[:, SL:], xt[:, :SL]], axis=1)
        in_maps.append(
            {
                "xt": np.ascontiguousarray(xt.astype(NPBF)),
                "wq2": wq2,
                "wk2": wk2,
                "wvt": wvt,
                "wot": wot,
                "bo": bo2,
                "sel": sel,
            }
        )
    return in_maps


LAST_EXEC_NS = None
LAST_RESULTS = None


def _ensure_ntff_hook_importable():
    """bass_utils imports antenv.axon_hooks when tracing is requested (e.g.
    BASS_TRACE=1 in the environment).  The RL container's antenv stub lacks
    that module; register a no-op fallback so tracing degrades gracefully
    instead of crashing.  If a real antenv.axon_hooks exists, do nothing."""
    import sys
    import types

    try:
        import antenv.axon_hooks  # noqa: F401

        return
    except ImportError:
        pass
    try:
        import antenv

        mod = types.ModuleType("antenv.axon_hooks")
        _hook = [None]
        mod.set_axon_ntff_profile_hook = lambda h: _hook.__setitem__(0, h)
        mod.get_axon_ntff_profile_hook = lambda: _hook[0]
        sys.modules["antenv.axon_hooks"] = mod
        antenv.axon_hooks = mod
        try:
            from trn_agent_boot.trn_boot import _ntff_profile_via_ctypes

            mod.set_axon_ntff_profile_hook(
                _ntff_profile_via_ctypes("/opt/axon/libaxon_pjrt.so")
            )
        except Exception:
            pass
    except Exception:
        pass


def _run(in_maps, trace=False):
    global LAST_EXEC_NS, LAST_RESULTS
    _ensure_ntff_hook_importable()
    if "nc" not in _CACHE:
        _CACHE["nc"] = build()
    res = run_bass_kernel_spmd(
        _CACHE["nc"],
        in_maps,
        core_ids=list(range(NCORES)),
        trace=trace,
    )
    LAST_RESULTS = res
    LAST_EXEC_NS = res.exec_time_ns
    return res


def kernel(X, Wq, Wk, Wv, Wo, bo):
    in_maps = _prep_inputs(X, Wq, Wk, Wv, Wo, bo)
    res = _run(in_maps, trace=bool(int(os.environ.get("KERNEL_TRACE", "0"))))
    out = np.empty((B, S, E), np.float32)
    for c in range(NCORES):
        b, sh = c // 2, c % 2
        out[b, sh * SL : (sh + 1) * SL, :] = res.results[c]["out"]
    return out



# revision 25
# speedup vs baseline: 1.0675x; 1.0151x over previous
"""Multi-headed attention kernel for 8 Trainium2 NeuronCores.

Problem: B=4, S=2048, E=1024, H=16, D=64 (torch-convention Linears, no bias
on q/k/v, bias on output projection).

Sharding: core c handles (batch b = c//2, query half sh = c%2).  Each core
computes Q for its 1024 query rows, K/V for the full 2048 keys of its batch
(duplicated across the pair of cores sharing a batch -- cheaper than any
cross-core collective), all 16 heads of attention for its rows, and the
output projection + bias.  Zero collectives.

Layout (feature dim on partitions; scores computed transposed):
  qT[f, q]  = sum_e WqT[e, f] * XT[e, q]          kT[f, s] likewise
  V[s, f]   = sum_e XT[e, s-chunk] * WvT[e, f]    (natural layout)
  scoresT[k, q] = sum_d kT[h*64+d, kc] * qT[h*64+d, q]    (K=64 matmuls,
      head pair packed in complementary PE row groups)
  EX = exp(scoresT / 8)            (ACT engine, PSUM -> SBUF bf16)
  ctxT_aug[m, q] = sum_k Vaug[k, h*65+m] * EX[k, q]   m in 0..64; V carries
      a ones column per head, so row 64 of the accumulation = softmax
      denominators (ones-column trick, M=65 matmuls)
  ctxT_norm = ctxT * bcast(1/denom)   (K=1 matmul broadcast of the
      denominator row + one-shot approx reciprocal on the DVE)
  out[s, e] = sum_f ctxT_norm[f, s-chunk] * WoT[f, e] + ones.T @ bo

Scheduling: the attention loop is ACT-bound (256 exp calls of [128,1024]).
Q/K/V projection work for head pair p+1 is emitted as "filler" inside pair
p's key-chunk loop so the PE stays dense while ACT streams exps (keeps the
HAM clock gate warm).  PSUM: scores 2x[128,1024] + ctx 3x[65,512] +
proj/bcast 1 = 8 banks.
"""

import os

import numpy as np
import ml_dtypes

import concourse.bass as bass
from concourse import bacc
import concourse.mybir as mybir
import concourse.tile as tile
from concourse.bass_utils import run_bass_kernel_spmd

B, S, E, H = 4, 2048, 1024, 16
D = E // H  # 64
P = 128
SL = S // 2     # local query rows per core (1024)
NCORES = 8
EC = E // P     # 8 e-chunks
FC = E // P     # 8 feature chunks
SC = S // P     # 16 s-chunks (V natural layout)
KC = S // P     # 16 key chunks (scores partition dim)
QB = SL // 512  # 2 query blocks of 512

F32 = mybir.dt.float32
F32R = mybir.dt.float32r
BF16 = mybir.dt.bfloat16
EXPF = mybir.ActivationFunctionType.Exp
NPBF = ml_dtypes.bfloat16

_CACHE = {}


def build():
    nc = bacc.Bacc(
        "TRN2",
        target_bir_lowering=False,
        debug=False,
        num_devices=NCORES,
    )

    xt_d = nc.dram_tensor("xt", [E, S], BF16, kind="ExternalInput").ap()
    # wq2/wk2 are host-pretiled: row fc*128+p, col ec*128+c  =  W.T[ec*128+p,
    # fc*128+c], so one contiguous [128, E] DMA delivers all 8 lhsT slices
    # for feature chunk fc.
    wq2_d = nc.dram_tensor("wq2", [E, E], BF16, kind="ExternalInput").ap()
    wk2_d = nc.dram_tensor("wk2", [E, E], BF16, kind="ExternalInput").ap()
    wvt_d = nc.dram_tensor("wvt", [E, E], BF16, kind="ExternalInput").ap()
    wot_d = nc.dram_tensor("wot", [E, E], BF16, kind="ExternalInput").ap()
    bo_d = nc.dram_tensor("bo", [1, E], BF16, kind="ExternalInput").ap()
    # one-hot selector rows: sel[r, r*64:(r+1)*64] = 1 (denominator broadcast)
    sel_d = nc.dram_tensor("sel", [4, 4 * D], F32R, kind="ExternalInput").ap()
    out_d = nc.dram_tensor("out", [SL, E], F32, kind="ExternalOutput").ap()

    with tile.TileContext(nc) as tc:
     with tc.tile_pool(name="persist", bufs=1) as persist:
        qt_sb = persist.tile([P, FC, SL], BF16, tag="qt")
        kt_sb = persist.tile([P, FC, S], BF16, tag="kt")
        DA = D + 1  # head dim + ones column
        vaug_sb = persist.tile([P, SC, H * DA], BF16, tag="vaug")
        vview = vaug_sb.rearrange("p c (h d) -> p c h d", d=DA)
        nc.vector.memset(vview[:, :, :, D : D + 1], 1.0)
        ctxt_sb = persist.tile([P, FC, SL], BF16, tag="ctxt")

        ones_bf = persist.tile([1, P], BF16, tag="ones_bf")   # bias matmul lhsT
        nc.vector.memset(ones_bf[:], 1.0)
        sel_sb = persist.tile([4, 4 * D], F32R, tag="sel")
        nc.sync.dma_start(out=sel_sb[:], in_=sel_d[:])

        from contextlib import ExitStack

        with (
            tc.tile_pool(name="wvp", bufs=8) as wvpool,
            tc.tile_pool(name="wqkp", bufs=5) as wqkpool,
            tc.tile_pool(name="expp", bufs=8) as exppool,
            tc.tile_pool(name="smallp", bufs=5) as smallpool,
        ):
            _xstack = ExitStack()
            _ostack = ExitStack()
            xpool = _xstack.enter_context(tc.tile_pool(name="xp", bufs=1))
            x_sb = xpool.tile([P, EC, S], BF16, tag="x")
            wv = []

            def load_wfc(w_dram, fc):
                """One [128, E] tile holding all 8 lhsT slices for chunk fc."""
                t = wqkpool.tile([P, E], BF16, tag="wqk", name="wqk")
                nc.sync.dma_start(out=t[:], in_=w_dram[fc * P : (fc + 1) * P, :])
                return t

            # ---- projection group emitters (8 accumulating MMs + 1 cast) ----
            def q_group(pool, wq_t, fc, qb):
                ps = pool.tile([P, 512], F32, tag="pj", name="pj")
                for ec in range(EC):
                    nc.tensor.matmul(
                        ps[:],
                        wq_t[:, ec * P : (ec + 1) * P],
                        x_sb[:, ec, qb * 512 : qb * 512 + 512],
                        start=(ec == 0),
                        stop=(ec == EC - 1),
                    )
                nc.vector.tensor_copy(
                    out=qt_sb[:, fc, qb * 512 : qb * 512 + 512], in_=ps[:]
                )

            def k_group(pool, wk_t, fc, kb):
                ps = pool.tile([P, 512], F32, tag="pj", name="pj")
                for ec in range(EC):
                    nc.tensor.matmul(
                        ps[:],
                        wk_t[:, ec * P : (ec + 1) * P],
                        x_sb[:, ec, kb * 512 : kb * 512 + 512],
                        start=(ec == 0),
                        stop=(ec == EC - 1),
                    )
                nc.vector.tensor_copy(
                    out=kt_sb[:, fc, kb * 512 : kb * 512 + 512], in_=ps[:]
                )

            def v_group(pool, sc, fb):
                ps = pool.tile([P, 512], F32, tag="pj", name="pj")
                for ec in range(EC):
                    nc.tensor.matmul(
                        ps[:],
                        x_sb[:, ec, sc * P : (sc + 1) * P],
                        wv[ec][:, fb * 512 : fb * 512 + 512],
                        start=(ec == 0),
                        stop=(ec == EC - 1),
                    )
                vv = vaug_sb[:, sc, :].rearrange("p (h d) -> p h d", d=DA)
                nc.vector.tensor_copy(
                    out=vv[:, fb * 8 : (fb + 1) * 8, 0:D],
                    in_=ps.rearrange("p (h d) -> p h d", d=D),
                )

            # ---------------- upfront: just enough for pair 0 ----------------
            # W chunk-0 tiles go on the Sync DMA queue; X streams in parallel
            # on the (otherwise idle) GpSimd DMA queue, Wv after it on Sync.
            wq_sl = load_wfc(wq2_d, 0)
            wk_sl = load_wfc(wk2_d, 0)
            for ec in range(EC):
                for hx in range(2):
                    nc.gpsimd.dma_start(
                        out=x_sb[:, ec, hx * 1024 : (hx + 1) * 1024],
                        in_=xt_d[ec * P : (ec + 1) * P, hx * 1024 : (hx + 1) * 1024],
                    )
            for ec in range(EC):
                t = wvpool.tile([P, E], BF16, tag="wv", name="wv")
                nc.sync.dma_start(out=t[:], in_=wvt_d[ec * P : (ec + 1) * P, :])
                wv.append(t)
            with tc.tile_pool(name="psum_u", bufs=6, space="PSUM") as psum_u:
                # advance all 6 Q/K accumulation groups together per arriving
                # X chunk: each 1.6us chunk DMA feeds ~1.6us of matmuls, so
                # the PE ramps with the DMA stream instead of stalling on the
                # last chunk of each group.
                psq = [
                    psum_u.tile([P, 512], F32, tag="pj", name="pj")
                    for _ in range(QB)
                ]
                psk = [
                    psum_u.tile([P, 512], F32, tag="pj", name="pj")
                    for _ in range(4)
                ]
                for ec in range(EC):
                    for qb in range(QB):
                        nc.tensor.matmul(
                            psq[qb][:],
                            wq_sl[:, ec * P : (ec + 1) * P],
                            x_sb[:, ec, qb * 512 : qb * 512 + 512],
                            start=(ec == 0),
                            stop=(ec == EC - 1),
                        )
                    for kb in range(4):
                        nc.tensor.matmul(
                            psk[kb][:],
                            wk_sl[:, ec * P : (ec + 1) * P],
                            x_sb[:, ec, kb * 512 : kb * 512 + 512],
                            start=(ec == 0),
                            stop=(ec == EC - 1),
                        )
                for qb in range(QB):
                    nc.vector.tensor_copy(
                        out=qt_sb[:, 0, qb * 512 : qb * 512 + 512], in_=psq[qb][:]
                    )
                for kb in range(4):
                    nc.vector.tensor_copy(
                        out=kt_sb[:, 0, kb * 512 : kb * 512 + 512], in_=psk[kb][:]
                    )
                # only V for the first few key chunks upfront; the rest are
                # produced inside pair 0's first kc loop (consumption of
                # vaug[sc] starts at kc==sc, so production stays ahead while
                # the exp stream hides the PE cost)
                for sc in range(4):
                    v_group(psum_u, sc, 0)

            # ---------------- pair loop ----------------
            # Emission-order = scheduler priority.  The kc loop is emitted
            # first (scores/exp/ctx only); filler projections for pair fc+1
            # are emitted AFTER it, so the priority heap interleaves filler
            # MMs into PE slack at single-MM granularity without ever
            # delaying a ready scores MM (they outrank the fillers).
            wot_t = []
            bo_sb = None
            with (
                tc.tile_pool(name="psum_sc", bufs=2, space="PSUM") as psum_sc,
                tc.tile_pool(name="psum_cx", bufs=2, space="PSUM") as psum_cx,
                tc.tile_pool(name="psum_pj", bufs=2, space="PSUM") as psum_pj,
            ):
                pending_norms = []
                wnext = {}  # nfc -> (wk_t, wq_t) with K0/Q0 already emitted
                for fc in range(FC):
                    hA, hB = 2 * fc, 2 * fc + 1
                    for qb in range(QB):
                        ctx_ps = {
                            hh: psum_cx.tile([DA, 512], F32, tag="ctx", name="ctx")
                            for hh in (0, 1)
                        }
                        for kc in range(KC):
                            with tc.high_priority(offset=1 << 20):
                                sc_ps = psum_sc.tile(
                                    [P, 1024], F32, tag="sc", name="sc"
                                )
                                for hh, h in ((0, hA), (1, hB)):
                                    po = hh * D
                                    nc.tensor.matmul(
                                        sc_ps[:, hh * 512 : hh * 512 + 512],
                                        kt_sb[
                                            po : po + D, fc, kc * P : (kc + 1) * P
                                        ],
                                        qt_sb[
                                            po : po + D,
                                            fc,
                                            qb * 512 : qb * 512 + 512,
                                        ],
                                        start=True,
                                        stop=True,
                                    )
                                ex = exppool.tile(
                                    [P, 1024], BF16, tag="exp", name="exp"
                                )
                                nc.scalar.activation(
                                    ex[:], sc_ps[:], EXPF, scale=0.125
                                )
                                for hh, h in ((0, hA), (1, hB)):
                                    # ctx + denom row via ones column (M=65)
                                    nc.tensor.matmul(
                                        ctx_ps[hh][0:DA, :],
                                        vaug_sb[:, kc, h * DA : (h + 1) * DA],
                                        ex[:, hh * 512 : hh * 512 + 512],
                                        start=(kc == 0),
                                        stop=(kc == KC - 1),
                                    )
                            if fc == 0 and qb == 0 and kc < SC - 4:
                                # stream the remaining pair-0 V chunks; the
                                # exp pipeline (exppool depth 8) absorbs the
                                # ctx lag while these run in PE slack
                                v_group(psum_pj, kc + 4, 0)

                        # ---- normalize: ctxt = ctx * bcast(1/denom) ----
                        # den extraction + ctxt copy free the ctx PSUM banks
                        # for the next qb -> keep them ahead of filler CASTs
                        # on the DVE queue.
                        with tc.high_priority(offset=1 << 20):
                            dens = []
                            for hh in (0, 1):
                                den = smallpool.tile(
                                    [1, 512], F32R, tag="den", name="den"
                                )
                                nc.vector.tensor_copy(
                                    out=den[:], in_=ctx_ps[hh][D : D + 1, :]
                                )
                                dens.append(den)
                            # release ctx banks: unnormalized bf16 into ctxt
                            for hh in (0, 1):
                                nc.vector.tensor_copy(
                                    out=ctxt_sb[
                                        hh * D : (hh + 1) * D,
                                        fc,
                                        qb * 512 : qb * 512 + 512,
                                    ],
                                    in_=ctx_ps[hh][0:D, :],
                                )

                        def _norm(dens=dens, fc=fc, qb=qb):
                            rec = smallpool.tile(
                                [P, 512], F32, tag="rec", name="rec"
                            )
                            for hh in (0, 1):
                                bc_ps = psum_pj.tile(
                                    [P, 512], F32, tag="pj", name="pj"
                                )
                                nc.tensor.matmul(
                                    bc_ps[0:D, :],
                                    sel_sb[0:1, 0:D],
                                    dens[hh][:],
                                    start=True,
                                    stop=True,
                                )
                                if hh == 0:
                                    nc.vector.reciprocal_approx_fast(
                                        out=rec[0:D, :], in_=bc_ps[0:D, :]
                                    )
                                else:
                                    # approx recip can't shift partitions; recip
                                    # at base 0 then cross-partition copy
                                    rtmp = smallpool.tile(
                                        [D, 512], F32, tag="rtmp", name="rtmp"
                                    )
                                    nc.vector.reciprocal_approx_fast(
                                        out=rtmp[:], in_=bc_ps[0:D, :]
                                    )
                                    nc.vector.tensor_copy(
                                        out=rec[D : 2 * D, :], in_=rtmp[:]
                                    )
                            dst = ctxt_sb[:, fc, qb * 512 : qb * 512 + 512]
                            nc.vector.tensor_mul(out=dst, in0=dst, in1=rec[:])

                        # Defer the norm (bcast MM + recip/mul chain): its
                        # sparse PE pattern would cool the HAM clock gate if
                        # it ran at an fc boundary; drained later it fills
                        # loop slack instead.  The last pair needs immediate
                        # norms for the overlapped output projection.
                        if fc < FC - 1:
                            pending_norms.append(_norm)
                        else:
                            _norm()
                        if fc == FC - 1:
                            # overlap the output projection for this qb's
                            # rows with the remaining attention work (uses
                            # the pj PSUM ring as filler-priority MMs)
                            for sc in range(qb * 4, qb * 4 + 4):
                                ot = outpool.tile([P, E], F32, tag="out", name="out")
                                for eb in range(2):
                                    ps = psum_pj.tile(
                                        [P, 512], F32, tag="pj", name="pj"
                                    )
                                    for fcc in range(FC):
                                        nc.tensor.matmul(
                                            ps[:],
                                            ctxt_sb[:, fcc, sc * P : (sc + 1) * P],
                                            wot_t[fcc][
                                                :, eb * 512 : eb * 512 + 512
                                            ],
                                            start=(fcc == 0),
                                            stop=False,
                                        )
                                    nc.tensor.matmul(
                                        ps[:],
                                        ones_bf[:],
                                        bo_sb[:, eb * 512 : eb * 512 + 512],
                                        start=False,
                                        stop=True,
                                    )
                                    nc.vector.tensor_copy(
                                        out=ot[:, eb * 512 : eb * 512 + 512],
                                        in_=ps[:],
                                    )
                                nc.sync.dma_start(
                                    out=out_d[sc * P : (sc + 1) * P, :], in_=ot[:]
                                )

                    # ---- fillers: prep pair fc+1 (emitted after the kc
                    # loop so every in-loop MM outranks them).  Hard deps of
                    # fc+1's first scores (K kb0, Q qb0) first; V groups and
                    # deferred norms spill harmlessly into fc+1's slack.
                    if fc + 1 < FC:
                        nfc = fc + 1
                        if nfc in wnext:
                            wk_n, wq_n = wnext.pop(nfc)
                        else:  # fc==0: fc1's critical pair not prepped yet
                            wk_n = load_wfc(wk2_d, nfc)
                            wq_n = load_wfc(wq2_d, nfc)
                            k_group(psum_pj, wk_n, nfc, 0)
                            q_group(psum_pj, wq_n, nfc, 0)
                        for kb in range(1, 4):
                            k_group(psum_pj, wk_n, nfc, kb)
                        q_group(psum_pj, wq_n, nfc, 1)
                        # two-ahead: the critical first-scores deps (K kb0,
                        # Q qb0) of pair fc+2 get a full extra span of slack
                        if fc + 2 < FC:
                            nnfc = fc + 2
                            wk_2 = load_wfc(wk2_d, nnfc)
                            wq_2 = load_wfc(wq2_d, nnfc)
                            k_group(psum_pj, wk_2, nnfc, 0)
                            q_group(psum_pj, wq_2, nnfc, 0)
                            wnext[nnfc] = (wk_2, wq_2)
                        if fc < 4:
                            for sc in range(4 * fc, 4 * fc + 4):
                                v_group(psum_pj, sc, 1)
                    while pending_norms:
                        pending_norms.pop(0)()
                    if fc == 4:
                        # prefetch Wo/bo into the (now dead) wv slots so the
                        # output projection never waits on DMA
                        for fcc in range(FC):
                            t = wvpool.tile([P, E], BF16, tag="wv", name="wv")
                            nc.sync.dma_start(
                                out=t[:], in_=wot_d[fcc * P : (fcc + 1) * P, :]
                            )
                            wot_t.append(t)
                        bo_sb = persist.tile([1, E], BF16, tag="bo")
                        nc.sync.dma_start(out=bo_sb[:], in_=bo_d[:])
                    if fc == FC - 2:
                        # X is dead once pair-7 prep is done; free its SBUF
                        # and open the output staging pool in the freed space
                        _xstack.close()
                        outpool = _ostack.enter_context(
                            tc.tile_pool(name="outp", bufs=2)
                        )
            _ostack.close()

    nc.compile()
    return nc


def _tile_wfc(wt):
    """Pre-tile W.T so chunk fc's 8 lhsT slices are one contiguous row-block:
    out[fc*128+p, ec*128+c] = wt[ec*128+p, fc*128+c]."""
    a = wt.reshape(EC, P, FC, P).transpose(2, 1, 0, 3)
    return np.ascontiguousarray(a.reshape(FC * P, E))


def _prep_inputs(X, Wq, Wk, Wv, Wo, bo):
    X = np.asarray(X, dtype=np.float32)
    wqt = np.ascontiguousarray(np.asarray(Wq, np.float32).T).astype(NPBF)
    wkt = np.ascontiguousarray(np.asarray(Wk, np.float32).T).astype(NPBF)
    wq2 = _tile_wfc(wqt)
    wk2 = _tile_wfc(wkt)
    wvt = np.ascontiguousarray(np.asarray(Wv, np.float32).T).astype(NPBF)
    wot = np.ascontiguousarray(np.asarray(Wo, np.float32).T).astype(NPBF)
    bo2 = np.asarray(bo, np.float32).reshape(1, E).astype(NPBF)
    sel = np.zeros((4, 4 * D), np.float32)
    for r in range(4):
        sel[r, r * D : (r + 1) * D] = 1.0

    in_maps = []
    for c in range(NCORES):
        b, sh = c // 2, c % 2
        xt = np.ascontiguousarray(X[b].T)  # [E, S]
        if sh == 1:  # rotate so the local query half comes first
            xt = np.concatenate([xt[:, SL:], xt[:, :SL]], axis=1)
        in_maps.append(
            {
                "xt": np.ascontiguousarray(xt.astype(NPBF)),
                "wq2": wq2,
                "wk2": wk2,
                "wvt": wvt,
                "wot": wot,
                "bo": bo2,
                "sel": sel,
            }
        )
    return in_maps


LAST_EXEC_NS = None
LAST_RESULTS = None


def _ensure_ntff_hook_importable():
    """bass_utils imports antenv.axon_hooks when tracing is requested (e.g.
    BASS_TRACE=1 in the environment).  The RL container's antenv stub lacks
    that module; register a no-op fallback so tracing degrades gracefully
    instead of crashing.  If a real antenv.axon_hooks exists, do nothing."""
    import sys
    import types

    try:
        import antenv.axon_hooks  # noqa: F401

        return
    except ImportError:
        pass
    try:
        import antenv

        mod = types.ModuleType("antenv.axon_hooks")
        _hook = [None]
        mod.set_axon_ntff_profile_hook = lambda h: _hook.__setitem__(0, h)
        mod.get_axon_ntff_profile_hook = lambda: _hook[0]
        sys.modules["antenv.axon_hooks"] = mod
        antenv.axon_hooks = mod
        try:
            from trn_agent_boot.trn_boot import _ntff_profile_via_ctypes

            mod.set_axon_ntff_profile_hook(
                _ntff_profile_via_ctypes("/opt/axon/libaxon_pjrt.so")
            )
        except Exception:
            pass
    except Exception:
        pass


def _run(in_maps, trace=False):
    global LAST_EXEC_NS, LAST_RESULTS
    _ensure_ntff_hook_importable()
    if "nc" not in _CACHE:
        _CACHE["nc"] = build()
    res = run_bass_kernel_spmd(
        _CACHE["nc"],
        in_maps,
        core_ids=list(range(NCORES)),
        trace=trace,
    )
    LAST_RESULTS = res
    LAST_EXEC_NS = res.exec_time_ns
    return res


def kernel(X, Wq, Wk, Wv, Wo, bo):
    in_maps = _prep_inputs(X, Wq, Wk, Wv, Wo, bo)
    res = _run(in_maps, trace=bool(int(os.environ.get("KERNEL_TRACE", "0"))))
    out = np.empty((B, S, E), np.float32)
    for c in range(NCORES):
        b, sh = c // 2, c % 2
        out[b, sh * SL : (sh + 1) * SL, :] = res.results[c]["out"]
    return out

